# revision 13
# baseline (speedup 1.0000x reference)
"""AR(64) sampling kernel for Trainium2 (8 NeuronCores, batch-sharded).

Problem: x_t = sum_k c_k x_{t-64+k} + sigma * eps_t over 4096 steps for
16384 independent batch rows (64 lags).

Approach: the recurrence is linear, so a block of 64 consecutive outputs
is an exact linear function of (the previous 64 outputs, the block's 64
noise values):

    y_block[i, b] = sum_r AS[i,r] * state[r, b] + sum_j sigma*AE[i,j] * eps[j, b]
                  = (W.T @ [state; eps])[i, b]      with W : [128, 64]

W is built on the host from the coefficients by running the recurrence
with unit initial conditions / unit impulses (exact linear algebra, a
few thousand host flops). On device, each core processes 2048 batch
rows (time-major layout) as a chain of 64 blocks x 4 batch-chunks of
512: one K=128 float32r matmul per (block, chunk), a PSUM->SBUF copy of
the result (which is simultaneously the next block's state rows and the
DMA-out staging), and streaming DMA of noise in / outputs out.

DMA layout: noise / outputs live in DRAM as [64 lane, n_blocks, Bc]
(lane-major), so a single DMA covering QUAD consecutive blocks moves
QUAD*8KB of contiguous DRAM per SBUF partition - large descriptors keep
all 16 SDMA engines at line rate.
"""

import os
import sys

import numpy as np

_TRN_REPO = "/opt/trn_rl_repo"
if _TRN_REPO not in sys.path:
    sys.path.insert(0, _TRN_REPO)

_TB = 64  # time-block size == number of AR lags
_NCORES = 8
_QUAD = 4  # time blocks per mega tile / per DMA
# float32r runs the PE single-pass (4x fp32 matmul rate) at ~1.2e-4
# relative input rounding; fp32 is exact but 4 cycles/row on the PE.
_USE_F32R = True


def _build_weights(coefficients: np.ndarray, sigma: float) -> np.ndarray:
    """Exact [2n, n] block-transition weights from AR coefficients.

    Returns lhsT with lhsT.T @ [state; eps] = y_block, where state is the
    previous 64 outputs (oldest first) and eps the block's raw noise.
    """
    c = np.asarray(coefficients, dtype=np.float64)
    n = c.shape[0]
    assert n == _TB

    # AS[i, r] = d y_i / d state_r : simulate with window = unit vectors.
    win = np.eye(n, dtype=np.float64)  # rows: unit-state cases
    AS = np.empty((_TB, n), dtype=np.float64)
    for i in range(_TB):
        x = win @ c
        AS[i] = x
        win = np.concatenate([win[:, 1:], x[:, None]], axis=1)

    # AE[i, j] = d y_i / d eps_j : simulate unit impulses, zero init.
    win = np.zeros((_TB, n), dtype=np.float64)
    AE = np.empty((_TB, _TB), dtype=np.float64)
    for i in range(_TB):
        x = win @ c
        x[i] += 1.0
        AE[i] = x
        win = np.concatenate([win[:, 1:], x[:, None]], axis=1)

    W = np.concatenate([AS.T, float(sigma) * AE.T], axis=0)  # [2n, TB]
    return np.ascontiguousarray(W.astype(np.float32))


def blocked_numpy(initial_values, coefficients, log_noise_std, noise):
    """Host-side blocked simulation (same math the device runs); for testing."""
    sigma = float(np.exp(np.float64(np.asarray(log_noise_std))))
    W = _build_weights(coefficients, sigma)
    B, T = noise.shape
    y = np.empty((B, T), np.float32)
    state = np.asarray(initial_values, np.float32)
    for b in range(T // _TB):
        rhs = np.concatenate([state.T, noise[:, b * _TB:(b + 1) * _TB].T], axis=0)
        out = (W.T @ rhs.astype(np.float32)).astype(np.float32)  # [TB, B]
        y[:, b * _TB:(b + 1) * _TB] = out.T
        state = out.T
    return y


def _round_f32r(x: np.ndarray) -> np.ndarray:
    """Round fp32 values to the fp32r matmul datapath precision (mantissa
    rounded at bit 12), round-to-nearest-even - bit-exact vs walrus's
    fp32_to_fp32r."""
    u = np.ascontiguousarray(x, np.float32).view(np.uint32)
    low = u & np.uint32(0xFFF)
    base = u >> np.uint32(12)
    add = (low > 0x800) | ((low == 0x800) & ((base & 1) == 1))
    r = ((base + add.astype(np.uint32)) << np.uint32(12)).astype(np.uint32)
    return r.view(np.float32)


def _build_nc(T: int, Bc: int, chunk: int, mega_bufs: int = 4):
    """Build the per-core Bass/Tile program.

    DRAM tensors (all fp32r bytes == fp32):
      noise_d [64, nb, Bc]  - lane-major noise (lane = offset within block)
      init_t  [64, Bc]      - initial window, lane-major (oldest first)
      w       [128, 64]     - block-transition weights (lhsT)
      y_d     [64, nb, Bc]  - lane-major outputs
    """
    from concourse import bacc
    import concourse.mybir as mybir
    from concourse.tile import TileContext

    assert T % (_TB * _QUAD) == 0 and Bc % chunk == 0
    nb = T // _TB
    ntiles = nb // _QUAD
    nchunks = Bc // chunk

    nc = bacc.Bacc("TRN2", target_bir_lowering=False, debug=False)
    f32 = mybir.dt.float32
    f32r = mybir.dt.float32r if _USE_F32R else f32
    noise_d = nc.dram_tensor("noise_d", [_TB, nb, Bc], f32r, kind="ExternalInput")
    init_t = nc.dram_tensor("init_t", [_TB, Bc], f32r, kind="ExternalInput")
    w = nc.dram_tensor("w", [2 * _TB, _TB], f32r, kind="ExternalInput")
    y_d = nc.dram_tensor("y_d", [_TB, nb, Bc], f32r, kind="ExternalOutput")

    with TileContext(nc) as tc:
        with tc.tile_pool(name="wpool", bufs=1) as wpool, \
             tc.tile_pool(name="mega", bufs=mega_bufs) as megapool, \
             tc.tile_pool(name="ps", bufs=2, space="PSUM") as pspool:
            wt = wpool.tile([2 * _TB, _TB], f32r)
            nc.sync.dma_start(out=wt[:, :], in_=w[:, :])

            def alloc_mega(g):
                t = megapool.tile([2 * _TB, _QUAD * Bc], f32r, tag="mega",
                                  name="mega")
                if g < ntiles:
                    # noise for blocks QUAD*g .. QUAD*g+3: one DMA, QUAD*8KB
                    # contiguous DRAM per partition.
                    nc.sync.dma_start(
                        out=t[_TB:, :].rearrange("p (q c) -> p q c", q=_QUAD),
                        in_=noise_d[:, _QUAD * g:_QUAD * (g + 1), :],
                    )
                return t

            # Mega tile g holds: rows 64:128 = noise of blocks 4g..4g+3
            # (col group j = block 4g+j); rows 0:64 col group j = state of
            # block 4g+j (written by the previous block's PSUM copy), which
            # is also the output of block 4g+j-1 -> store staging.
            megas = {0: alloc_mega(0), 1: alloc_mega(1)}
            nc.sync.dma_start(out=megas[0][0:_TB, 0:Bc], in_=init_t[:, :])

            for g in range(ntiles):
                cur = megas[g]
                if g + 2 <= ntiles:
                    megas[g + 2] = alloc_mega(g + 2)
                for j in range(_QUAD):
                    b = _QUAD * g + j
                    goff = j * Bc
                    pss = []
                    for c in range(nchunks):
                        cs = slice(goff + c * chunk, goff + (c + 1) * chunk)
                        ps = pspool.tile([_TB, chunk], f32, tag=f"ps{c}",
                                         name=f"ps{c}")
                        # float32r: single-pass PE matmul (4x fp32 rate).
                        nc.tensor.matmul(
                            out=ps[:, :], lhsT=wt[:, :], rhs=cur[:, cs],
                            start=True, stop=True,
                        )
                        pss.append(ps)
                    if b + 1 < nb:
                        dst_t = megas[(b + 1) // _QUAD]
                        doff = ((b + 1) % _QUAD) * Bc
                    else:
                        dst_t = megas[ntiles]  # tail tile, col 0
                        doff = 0
                    for c in range(nchunks):
                        dcs = slice(doff + c * chunk, doff + (c + 1) * chunk)
                        # Split PSUM->SBUF copies across DVE and ACT.
                        if c % 2 == 0:
                            nc.vector.tensor_copy(out=dst_t[0:_TB, dcs],
                                                  in_=pss[c][:, :])
                        else:
                            nc.scalar.copy(out=dst_t[0:_TB, dcs],
                                           in_=pss[c][:, :])
                # Store outputs staged in this tile's state rows: blocks
                # 4g-1 .. 4g+2 (col 0 of tile 0 is the initial window -
                # skip it). ACT HW-DGE ring; loads ride the SP ring.
                if g == 0:
                    nc.scalar.dma_start(
                        out=y_d[:, 0:_QUAD - 1, :],
                        in_=cur[0:_TB, Bc:].rearrange(
                            "p (q c) -> p q c", q=_QUAD - 1),
                    )
                else:
                    nc.scalar.dma_start(
                        out=y_d[:, _QUAD * g - 1:_QUAD * g + _QUAD - 1, :],
                        in_=cur[0:_TB, :].rearrange(
                            "p (q c) -> p q c", q=_QUAD),
                    )
            # Tail: block nb-1's output sits in the extra tile's col 0.
            nc.scalar.dma_start(
                out=y_d[:, nb - 1:nb, :],
                in_=megas[ntiles][0:_TB, 0:Bc].rearrange(
                    "p (q c) -> p q c", q=1),
            )

    nc.compile()
    return nc


def _shard_inputs(initial_values, coefficients, log_noise_std, noise):
    B, T = noise.shape
    Bc = B // _NCORES
    nb = T // _TB
    sigma = float(np.exp(np.float64(np.asarray(log_noise_std))))
    rnd = _round_f32r if _USE_F32R else (lambda x: np.ascontiguousarray(x, np.float32))
    W = rnd(_build_weights(coefficients, sigma))
    noise_tf = rnd(np.asarray(noise, np.float32).T)  # [T, B]
    init_tf = rnd(np.asarray(initial_values, np.float32).T)
    # lane-major: [T, B] -> [nb, 64, B] -> [64, nb, B]
    noise_lane = np.ascontiguousarray(
        noise_tf.reshape(nb, _TB, B).transpose(1, 0, 2))
    in_maps = []
    for i in range(_NCORES):
        cols = slice(i * Bc, (i + 1) * Bc)
        in_maps.append({
            "noise_d": np.ascontiguousarray(noise_lane[:, :, cols]),
            "init_t": np.ascontiguousarray(init_tf[:, cols]),
            "w": W,
        })
    return in_maps


def _run(initial_values, coefficients, log_noise_std, noise, trace=False):
    from concourse.bass_utils import run_bass_kernel_spmd

    B, T = noise.shape
    Bc = B // _NCORES
    chunk = 512 if Bc % 512 == 0 else Bc
    nc = _build_nc(T, Bc, chunk)
    in_maps = _shard_inputs(initial_values, coefficients, log_noise_std, noise)
    res = run_bass_kernel_spmd(
        nc, in_maps, core_ids=list(range(_NCORES)), trace=trace
    )
    nb = T // _TB
    # y_d [64, nb, Bc] lane-major -> [T, Bc]
    y_cores = [
        r["y_d"].transpose(1, 0, 2).reshape(T, Bc) for r in res.results
    ]
    y_t = np.concatenate(y_cores, axis=1)  # [T, B]
    out = np.ascontiguousarray(y_t.T)
    return out, res


def kernel(initial_values, coefficients, log_noise_std, noise, steps):
    steps = int(np.asarray(steps))
    noise = np.asarray(noise)
    assert noise.shape[1] == steps, (noise.shape, steps)
    out, _ = _run(initial_values, coefficients, log_noise_std, noise)
    return out
